# revision 40
# baseline (speedup 1.0000x reference)
"""Multi-head attention (B=2, S=2048, D=1024, H=16) on 8 Trainium2 NeuronCores.

Sharding: tensor-parallel on heads (4 groups of 4 heads) x data-parallel on
batch (2) -> 8 cores. Each core computes QKV projections for its head slice,
attention for its 4 heads, and a partial output projection; the host sums the
4 partials per batch element (the tensor-parallel allreduce) and adds bo.

v3 design: the softmax exp stream on the ACT engine is the pacing resource
(~135us); everything else hides under it.
- One flat stream of (head, q-block, kblock) steps: scores -> exp each step,
  attn@V lagging 3 steps behind, so head boundaries never stall the exp
  stream (the in-order PE queue always has ready work in front).
- Scores are computed transposed (ST[k,q] = K_h.T @ Q_h) per (head, kblock)
  into a 2-buffer x [128,1024] PSUM ring (4 banks) -> exp runs at FD=1024
  with no ping-pong stalls.  The two 512-wide score matmuls of each step run
  CONCURRENTLY on disjoint 64-row PE tiles (K/Q live in both partition
  halves via a partition-swapped copy), so dk=64 wastes nothing.
- attn@V accumulates po[65, 1024] in PSUM (2 banks); a ones-column appended
  to V makes row 64 the softmax denominators.  Normalize first copies po to
  SBUF (freeing the single po buffer early), then recip/broadcast/scale.
- QKV/O projections are [128,512]-output units on a 2-buffer PSUM pool
  (2 banks), interleaved into the stream to fill PE slack.  Prologue DMAs
  are spread across the sync/scalar/gpsimd queues (the ~625ns per-DMA issue
  cost on one queue would otherwise serialize the start).
"""

import numpy as np

import concourse.bass as bass  # noqa: F401
import concourse.tile as tile
from concourse import bacc, mybir
from concourse.bass_utils import run_bass_kernel_spmd

D_MODEL = 1024
NUM_HEADS = 16
DK = 64
B, S = 2, 2048
N_CORES = 8
GROUPS = 4                 # head groups (tensor parallel)
GW = D_MODEL // GROUPS     # 256 features per group = 4 heads
HPG = GROUPS               # heads per group = 4

F32 = mybir.dt.float32
F16 = mybir.dt.float16
EXPF = mybir.ActivationFunctionType.Exp
MULT = mybir.AluOpType.mult
ADD = mybir.AluOpType.add

LAG = 3                    # attn@V trails exp by this many stream steps


def _emit(nc, tc, ctx):
    P = 128
    xqT = nc.dram_tensor("xqT", [D_MODEL, S], F16, kind="ExternalInput")
    xkT = nc.dram_tensor("xkT", [D_MODEL, S], F16, kind="ExternalInput")
    xvT = nc.dram_tensor("xvT", [D_MODEL, S], F16, kind="ExternalInput")
    wqT = nc.dram_tensor("wqT", [D_MODEL, GW], F16, kind="ExternalInput")
    wkT = nc.dram_tensor("wkT", [D_MODEL, GW], F16, kind="ExternalInput")
    wvT = nc.dram_tensor("wvT", [D_MODEL, GW], F16, kind="ExternalInput")
    woT = nc.dram_tensor("woT", [GW, D_MODEL], F16, kind="ExternalInput")
    bq2 = nc.dram_tensor("bq2", [P, 2], F32, kind="ExternalInput")
    bk2 = nc.dram_tensor("bk2", [P, 2], F32, kind="ExternalInput")
    bvr = nc.dram_tensor("bvr", [1, GW], F32, kind="ExternalInput")
    out = nc.dram_tensor("out", [S, D_MODEL], F16, kind="ExternalOutput")

    consts = ctx.enter_context(tc.tile_pool(name="consts", bufs=1))
    persist = ctx.enter_context(tc.tile_pool(name="persist", bufs=1))
    xs = ctx.enter_context(tc.tile_pool(name="xs", bufs=3))
    xv_pool = ctx.enter_context(tc.tile_pool(name="xvs", bufs=3))
    sx = ctx.enter_context(tc.tile_pool(name="stexp", bufs=5))
    nrm = ctx.enter_context(tc.tile_pool(name="nrm", bufs=2))
    outp = ctx.enter_context(tc.tile_pool(name="outp", bufs=3))
    o1pool = ctx.enter_context(tc.tile_pool(name="o1pool", bufs=8))
    psA = ctx.enter_context(tc.tile_pool(name="psA", bufs=2, space="PSUM"))
    psB = ctx.enter_context(tc.tile_pool(name="psB", bufs=1, space="PSUM"))
    psP = ctx.enter_context(tc.tile_pool(name="psP", bufs=2, space="PSUM"))

    # ---- weights / constants (queue-balanced) ---------------------------
    wq_sb = consts.tile([P, 8, GW], F16)
    wk_sb = consts.tile([P, 8, GW], F16)
    wv_sb = consts.tile([P, 8, GW], F16)
    wo_sb = consts.tile([P, 2, D_MODEL], F16)
    bq_sb = consts.tile([P, 2], F32)
    bk_sb = consts.tile([P, 2], F32)
    bv_row = consts.tile([1, GW], F32)
    nc.sync.dma_start(wq_sb[:], wqT[:].rearrange("(c p) j -> p c j", p=P))
    nc.sync.dma_start(bq_sb[:], bq2[:])
    nc.scalar.dma_start(wk_sb[:], wkT[:].rearrange("(c p) j -> p c j", p=P))
    nc.scalar.dma_start(bk_sb[:], bk2[:])
    nc.gpsimd.dma_start(bv_row[:], bvr[:])
    bvb = consts.tile([P, GW], F32)
    nc.gpsimd.partition_broadcast(bvb[:], bv_row[:])

    # persistent activations.  QT[jc]: projected Q^T, partitions = the 128
    # features of chunk jc (head (jc,0) rows 0-63, head (jc,1) rows 64-127),
    # free dim = q.  QTw = partition-halves-swapped copy (for PE row-tiling).
    # After attention, OT overwrites QT's dead region (per-head rows).
    QT = [persist.tile([P, S], F16, name=f"QT{j}") for j in range(2)]
    QTw = [persist.tile([P, S], F16, name=f"QTw{j}") for j in range(2)]
    KT = [[persist.tile([P, 1024], F16, name=f"KT{j}_{sb}") for sb in range(2)]
          for j in range(2)]
    KTw = [[persist.tile([P, 1024], F16, name=f"KTw{j}_{sb}") for sb in range(2)]
           for j in range(2)]
    # V + ones column, per (kblock, head): lhsT of attn@V
    Vaug = persist.tile([P, 16, HPG, DK + 1], F16, name="Vaug")
    ones_f32 = consts.tile([P, 16, HPG], F32)
    nc.vector.memset(ones_f32[:], 1.0)
    # warm the ACT exp table during the DMA-bound prologue
    warm = consts.tile([1, 1], F32)
    nc.scalar.activation(out=warm[:], in_=ones_f32[0:1, 0, 0:1], func=EXPF)
    nc.vector.tensor_scalar_add(Vaug[:, :, :, DK], ones_f32[:], 0.0)

    xqT_r = xqT[:].rearrange("(c p) s -> p c s", p=P)
    xkT_r = xkT[:].rearrange("(c p) s -> p c s", p=P)
    xvT_r = xvT[:].rearrange("(c p) s -> p c s", p=P)

    # ---- projection work units ------------------------------------------
    def stage_x(x_r, sb, name, eng=None, split=2):
        """Stage one s-half of an input: 8 [128,1024] chunks in `split` DMAs."""
        eng = eng or nc.sync
        xt = xs.tile([P, 8, 1024], F16, tag="xs", name=f"x{name}{sb}")
        step = 8 // split
        for g in range(split):
            eng.dma_start(xt[:, g * step:(g + 1) * step, :],
                          x_r[:, g * step:(g + 1) * step,
                              sb * 1024:(sb + 1) * 1024])
        return xt

    def qk_mms(xt, w_sb, b_sb, dst, jc, sb, ns, full_s=False):
        """Projection matmuls + main drain for 512 s-columns of chunk jc.
        Returns a closure writing the partition-swapped copy (reads the same
        PSUM tile, so it must run before psP recycles this buffer)."""
        ps = psP.tile([P, 512], F32, tag="psP", name=f"ps{jc}{sb}{ns}")
        for i in range(8):
            nc.tensor.matmul(
                ps[:, :],
                w_sb[:, i, jc * P:(jc + 1) * P],
                xt[:, i, ns * 512:(ns + 1) * 512],
                start=(i == 0), stop=(i == 7),
            )
        c0 = (sb * 1024 if full_s else 0) + ns * 512
        sl = slice(c0, c0 + 512)
        nc.vector.tensor_scalar_add(dst[:, sl], ps[:], b_sb[:, jc:jc + 1])

        def swaps(dstw):
            nc.vector.tensor_scalar_add(dstw[64:128, sl], ps[0:64, :],
                                        b_sb[0:64, jc:jc + 1])
            nc.vector.tensor_scalar_add(dstw[0:64, sl], ps[64:128, :],
                                        b_sb[64:128, jc:jc + 1])
        return swaps

    def qk_unit(xt, w_sb, b_sb, dst, dstw, jc, sb, ns, full_s=False):
        qk_mms(xt, w_sb, b_sb, dst, jc, sb, ns, full_s)(dstw)

    def v_unit(kb):
        """V projection for kblock kb -> Vaug[:, kb, :, 0:64] (+bias)."""
        xvt = xv_pool.tile([P, 8, P], F16, tag="xv", name=f"xv{kb}")
        nc.gpsimd.dma_start(xvt[:], xvT_r[:, :, kb * P:(kb + 1) * P])
        pv = psP.tile([P, 512], F32, tag="psP", name=f"pv{kb}")
        for i in range(8):
            nc.tensor.matmul(
                pv[:, 0:GW], xvt[:, i, :], wv_sb[:, i, :],
                start=(i == 0), stop=(i == 7),
            )
        nc.vector.tensor_tensor(
            Vaug[:, kb, :, 0:DK],
            pv[:, 0:GW].rearrange("p (h d) -> p h d", h=HPG),
            bvb[:].rearrange("p (h d) -> p h d", h=HPG),
            ADD,
        )

    ot_tiles = {}

    def oproj_full(sc):
        """Complete O projection for s-chunk sc (both halves + store)."""
        ot = outp.tile([P, 1024], F16, tag="osb", name=f"ot{sc}")
        for ms in range(2):
            pso = psP.tile([P, 512], F32, tag="psP", name=f"pso{sc}{ms}")
            for hd in range(2):
                nc.tensor.matmul(
                    pso[:, :],
                    QT[hd][:, sc * P:(sc + 1) * P],
                    wo_sb[:, hd, ms * 512:(ms + 1) * 512],
                    start=(hd == 0), stop=(hd == 1),
                )
            nc.vector.tensor_copy(out=ot[:, ms * 512:(ms + 1) * 512],
                                  in_=pso[:, :])
        eng = nc.sync if sc % 2 == 0 else nc.gpsimd
        eng.dma_start(out[sc * P:(sc + 1) * P, :], ot[:])

    def oproj_part1(sc):
        """hd=0 half of the contraction for s-chunk sc -> SBUF staging."""
        ot = o1pool.tile([P, 1024], F32, tag="o1", name=f"o1_{sc}")
        ot_tiles[sc] = ot
        for ms in range(2):
            pso = psP.tile([P, 512], F32, tag="psP", name=f"p1_{sc}{ms}")
            nc.tensor.matmul(pso[:, :], QT[0][:, sc * P:(sc + 1) * P],
                             wo_sb[:, 0, ms * 512:(ms + 1) * 512],
                             start=True, stop=True)
            nc.vector.tensor_copy(out=ot[:, ms * 512:(ms + 1) * 512],
                                  in_=pso[:, :])

    def oproj_part2(sc):
        """hd=1 half + add + store for s-chunk sc."""
        ot1 = ot_tiles[sc]
        ot2 = outp.tile([P, 1024], F16, tag="osb2", name=f"o2_{sc}")
        for ms in range(2):
            pso = psP.tile([P, 512], F32, tag="psP", name=f"p2_{sc}{ms}")
            nc.tensor.matmul(pso[:, :], QT[1][:, sc * P:(sc + 1) * P],
                             wo_sb[:, 1, ms * 512:(ms + 1) * 512],
                             start=True, stop=True)
            nc.vector.tensor_tensor(ot2[:, ms * 512:(ms + 1) * 512], pso[:, :],
                                    ot1[:, ms * 512:(ms + 1) * 512], ADD)
        eng = nc.sync if sc % 2 == 0 else nc.gpsimd
        eng.dma_start(out[sc * P:(sc + 1) * P, :], ot2[:])

    # ---- attention pieces ----------------------------------------------
    _pst_box = [None]

    def scores_pair(h, qb, kb):
        """ST[k,q]: two concurrent 64-row-tile matmuls (home + swapped)."""
        jc, hp = h // 2, h % 2
        pr = 64 * hp
        prw = 64 - pr
        sb, kc = kb // 8, (kb % 8) * P
        pst = psA.tile([P, 1024], F32, tag="psA", name=f"pst{h}{qb}{kb}")
        _pst_box[0] = pst
        q0 = qb * 1024
        nc.tensor.matmul(
            pst[:, 0:512],
            KT[jc][sb][pr:pr + DK, kc:kc + P],
            QT[jc][pr:pr + DK, q0:q0 + 512],
            start=True, stop=True,
        )
        nc.tensor.matmul(
            pst[:, 512:1024],
            KTw[jc][sb][prw:prw + DK, kc:kc + P],
            QTw[jc][prw:prw + DK, q0 + 512:q0 + 1024],
            start=True, stop=True,
        )

    def attn_v(po, st, h, kb):
        for ns in range(2):
            nc.tensor.matmul(
                po[0:DK + 1, ns * 512:(ns + 1) * 512],
                Vaug[:, kb, h, :],
                st[:, ns * 512:(ns + 1) * 512],
                start=(kb == 0), stop=(kb == 15),
            )

    def normalize(po, h, qb):
        """Copy po out of PSUM (frees the single po buffer), then scale by
        the reciprocal of the denominators (row DK) and write OT into QT."""
        jc, hp = h // 2, h % 2
        pr = 64 * hp
        poc = nrm.tile([DK + 1, 1024], F32, tag="poc", name=f"poc{h}{qb}")
        dn = nrm.tile([1, 1024], F32, tag="dn", name=f"dn{h}{qb}")
        nc.vector.tensor_copy(out=poc[:], in_=po[0:DK + 1, :])
        nc.vector.tensor_copy(out=dn[:], in_=poc[DK:DK + 1, :])
        bc = nrm.tile([DK, 1024], F32, tag="bcast", name=f"bc{h}{qb}")
        nc.vector.reciprocal_approx_fast(bc[0:1, :], dn[:])
        nc.gpsimd.partition_broadcast(bc[:], bc[0:1, :])
        q0 = qb * 1024
        nc.vector.tensor_tensor(
            QT[jc][pr:pr + DK, q0:q0 + 1024], poc[0:DK, :], bc[:], MULT)

    # ---- prologue -------------------------------------------------------
    # xq0 on sync, xk0 on scalar (idle until the first exp), weights split.
    xq0 = stage_x(xqT_r, 0, "q", eng=nc.sync, split=8)
    xk0 = stage_x(xkT_r, 0, "k", eng=nc.scalar, split=8)
    sw_q0 = qk_mms(xq0, wq_sb, bq_sb, QT[0], 0, 0, 0, full_s=True)
    sw_q1 = qk_mms(xq0, wq_sb, bq_sb, QT[0], 0, 0, 1, full_s=True)
    sw_q0(QTw[0])
    sw_q1(QTw[0])
    sw_k0 = qk_mms(xk0, wk_sb, bk_sb, KT[0][0], 0, 0, 0)
    sw_k0(KTw[0][0])
    sw_k1 = qk_mms(xk0, wk_sb, bk_sb, KT[0][0], 0, 0, 1)
    sw_k1(KTw[0][0])
    wvT_r = wvT[:].rearrange("(c p) j -> p c j", p=P)
    for i in range(8):
        nc.scalar.dma_start(wv_sb[:, i, :], wvT_r[:, i, :])
    woT_r = woT[:].rearrange("(c p) m -> p c m", p=P)
    for cc in range(2):
        nc.sync.dma_start(wo_sb[:, cc, :], woT_r[:, cc, :])
    for kb in range(4):
        v_unit(kb)
    xk1 = stage_x(xkT_r, 1, "k1", eng=nc.gpsimd, split=2)

    # deferred projection / oproj work, as (slot -> closure) over the stream
    box = {}
    fills = {}

    def put(step, fn):
        fills.setdefault(step, []).append(fn)

    # u0 (h0,qb0): remaining V units inline + the sb1 K units it needs
    for kb in range(12):
        put(kb, lambda kb=kb: v_unit(kb + 4))
    put(1, lambda: qk_unit(xk1, wk_sb, bk_sb, KT[0][1], KTw[0][1], 0, 1, 0))
    put(4, lambda: qk_unit(xk1, wk_sb, bk_sb, KT[0][1], KTw[0][1], 0, 1, 1))
    # u1 (h1,qb0): Q/K for the jc=1 heads + stage xq sb1
    put(17, lambda: qk_unit(xq0, wq_sb, bq_sb, QT[1], QTw[1], 1, 0, 0,
                            full_s=True))
    put(19, lambda: qk_unit(xq0, wq_sb, bq_sb, QT[1], QTw[1], 1, 0, 1,
                            full_s=True))
    put(21, lambda: qk_unit(xk0, wk_sb, bk_sb, KT[1][0], KTw[1][0], 1, 0, 0))
    put(23, lambda: qk_unit(xk0, wk_sb, bk_sb, KT[1][0], KTw[1][0], 1, 0, 1))
    put(26, lambda: box.__setitem__(
        "xq1", stage_x(xqT_r, 1, "q1", eng=nc.gpsimd, split=2)))
    # u2 (h2,qb0): jc=1 sb1 K units + Q sb1 units
    put(33, lambda: qk_unit(xk1, wk_sb, bk_sb, KT[1][1], KTw[1][1], 1, 1, 0))
    put(35, lambda: qk_unit(xk1, wk_sb, bk_sb, KT[1][1], KTw[1][1], 1, 1, 1))
    put(38, lambda: qk_unit(box["xq1"], wq_sb, bq_sb, QT[0], QTw[0], 0, 1, 0,
                            full_s=True))
    put(41, lambda: qk_unit(box["xq1"], wq_sb, bq_sb, QT[0], QTw[0], 0, 1, 1,
                            full_s=True))
    put(44, lambda: qk_unit(box["xq1"], wq_sb, bq_sb, QT[1], QTw[1], 1, 1, 0,
                            full_s=True))
    put(47, lambda: qk_unit(box["xq1"], wq_sb, bq_sb, QT[1], QTw[1], 1, 1, 1,
                            full_s=True))
    # qb0 O-projection during u4,u5 (needs all qb0 heads normalized)
    for j, sc in enumerate(range(8)):
        put(68 + 3 * j, lambda sc=sc: oproj_full(sc))
    # qb1 O-projection: hd0 half during u6/u7, hd1 half in the tail
    for j, sc in enumerate(range(8, 16)):
        put(99 + 3 * j, lambda sc=sc: oproj_part1(sc))

    # ---- the flat attention stream -------------------------------------
    units = [(h, qb) for qb in (0, 1) for h in range(4)]
    seq = [(u, kb) for u in range(8) for kb in range(16)]
    po_of = {}
    sts = {}

    def do_lagged(i):
        u2, kb2 = seq[i]
        h2, qb2 = units[u2]
        if kb2 == 0:
            po_of[u2] = psB.tile([P, 1024], F32, tag="po", name=f"po{u2}")
        attn_v(po_of[u2], sts[(u2, kb2)], h2, kb2)
        del sts[(u2, kb2)]
        if kb2 == 15:
            normalize(po_of[u2], h2, qb2)

    for i, (u, kb) in enumerate(seq):
        h, qb = units[u]
        scores_pair(h, qb, kb)
        st = sx.tile([P, 1024], F16, tag="st", name=f"st{u}_{kb}")
        nc.scalar.activation(out=st[:], in_=_pst_box[0][:], func=EXPF,
                             scale=0.125)
        sts[(u, kb)] = st
        if i >= LAG:
            do_lagged(i - LAG)
        for fn in fills.pop(i, ()):
            fn()
    for i in range(len(seq) - LAG, len(seq)):
        do_lagged(i)
    # tail: finish qb1 O-projection
    for sc in range(8, 16):
        oproj_part2(sc)


_prog_cache = {}


def _build_program():
    if "nc" not in _prog_cache:
        from contextlib import ExitStack
        nc = bacc.Bacc("TRN2", target_bir_lowering=False)
        with tile.TileContext(nc) as tc:
            with ExitStack() as ctx:
                _emit(nc, tc, ctx)
        nc.compile()
        _prog_cache["nc"] = nc
    return _prog_cache["nc"]


def make_in_maps(query, key, value, Wq, bq, Wk, bk, Wv, bv, Wo, bo):
    query, key, value = (np.asarray(t, np.float32) for t in (query, key, value))
    Wq, Wk, Wv, Wo = (np.asarray(t, np.float32) for t in (Wq, Wk, Wv, Wo))
    bq, bk, bv = (np.asarray(t, np.float32) for t in (bq, bk, bv))
    xT = {b: {} for b in range(B)}
    for b in range(B):
        xT[b]["q"] = np.ascontiguousarray(query[b].T).astype(np.float16)
        xT[b]["k"] = np.ascontiguousarray(key[b].T).astype(np.float16)
        xT[b]["v"] = np.ascontiguousarray(value[b].T).astype(np.float16)
    in_maps = []
    for c in range(N_CORES):
        b, g = divmod(c, GROUPS)
        gs = slice(g * GW, (g + 1) * GW)
        in_maps.append({
            "xqT": xT[b]["q"], "xkT": xT[b]["k"], "xvT": xT[b]["v"],
            "wqT": np.ascontiguousarray(Wq[gs, :].T).astype(np.float16),
            "wkT": np.ascontiguousarray(Wk[gs, :].T).astype(np.float16),
            "wvT": np.ascontiguousarray(Wv[gs, :].T).astype(np.float16),
            "woT": np.ascontiguousarray(Wo[:, gs].T).astype(np.float16),
            "bq2": np.ascontiguousarray(bq[gs].reshape(2, 128).T),
            "bk2": np.ascontiguousarray(bk[gs].reshape(2, 128).T),
            "bvr": np.ascontiguousarray(bv[gs].reshape(1, GW)),
        })
    return in_maps


def run_on_hw(in_maps, trace=False, **kw):
    nc = _build_program()
    return run_bass_kernel_spmd(nc, in_maps, core_ids=list(range(N_CORES)),
                                trace=trace, **kw)


def kernel(query, key, value, Wq, bq, Wk, bk, Wv, bv, Wo, bo):
    in_maps = make_in_maps(query, key, value, Wq, bq, Wk, bk, Wv, bv, Wo, bo)
    res = run_on_hw(in_maps)
    out = np.zeros((B, S, D_MODEL), np.float32)
    for c in range(N_CORES):
        out[c // GROUPS] += res.results[c]["out"].astype(np.float32)
    out += np.asarray(bo, np.float32)
    return out



# revision 41
# speedup vs baseline: 1.0006x; 1.0006x over previous
"""Multi-head attention (B=2, S=2048, D=1024, H=16) on 8 Trainium2 NeuronCores.

Sharding: tensor-parallel on heads (4 groups of 4 heads) x data-parallel on
batch (2) -> 8 cores. Each core computes QKV projections for its head slice,
attention for its 4 heads, and a partial output projection; the host sums the
4 partials per batch element (the tensor-parallel allreduce) and adds bo.

v3 design: the softmax exp stream on the ACT engine is the pacing resource
(~135us); everything else hides under it.
- One flat stream of (head, q-block, kblock) steps: scores -> exp each step,
  attn@V lagging 3 steps behind, so head boundaries never stall the exp
  stream (the in-order PE queue always has ready work in front).
- Scores are computed transposed (ST[k,q] = K_h.T @ Q_h) per (head, kblock)
  into a 2-buffer x [128,1024] PSUM ring (4 banks) -> exp runs at FD=1024
  with no ping-pong stalls.  The two 512-wide score matmuls of each step run
  CONCURRENTLY on disjoint 64-row PE tiles (K/Q live in both partition
  halves via a partition-swapped copy), so dk=64 wastes nothing.
- attn@V accumulates po[65, 1024] in PSUM (2 banks); a ones-column appended
  to V makes row 64 the softmax denominators.  Normalize first copies po to
  SBUF (freeing the single po buffer early), then recip/broadcast/scale.
- QKV/O projections are [128,512]-output units on a 2-buffer PSUM pool
  (2 banks), interleaved into the stream to fill PE slack.  Prologue DMAs
  are spread across the sync/scalar/gpsimd queues (the ~625ns per-DMA issue
  cost on one queue would otherwise serialize the start).
"""

import numpy as np

import concourse.bass as bass  # noqa: F401
import concourse.tile as tile
from concourse import bacc, mybir
from concourse.bass_utils import run_bass_kernel_spmd

D_MODEL = 1024
NUM_HEADS = 16
DK = 64
B, S = 2, 2048
N_CORES = 8
GROUPS = 4                 # head groups (tensor parallel)
GW = D_MODEL // GROUPS     # 256 features per group = 4 heads
HPG = GROUPS               # heads per group = 4

F32 = mybir.dt.float32
F16 = mybir.dt.float16
EXPF = mybir.ActivationFunctionType.Exp
MULT = mybir.AluOpType.mult
ADD = mybir.AluOpType.add

LAG = 3                    # attn@V trails exp by this many stream steps


def _emit(nc, tc, ctx):
    P = 128
    xqT = nc.dram_tensor("xqT", [D_MODEL, S], F16, kind="ExternalInput")
    xkT = nc.dram_tensor("xkT", [D_MODEL, S], F16, kind="ExternalInput")
    xvT = nc.dram_tensor("xvT", [D_MODEL, S], F16, kind="ExternalInput")
    wqT = nc.dram_tensor("wqT", [D_MODEL, GW], F16, kind="ExternalInput")
    wkT = nc.dram_tensor("wkT", [D_MODEL, GW], F16, kind="ExternalInput")
    wvT = nc.dram_tensor("wvT", [D_MODEL, GW], F16, kind="ExternalInput")
    woT = nc.dram_tensor("woT", [GW, D_MODEL], F16, kind="ExternalInput")
    bq2 = nc.dram_tensor("bq2", [P, 2], F32, kind="ExternalInput")
    bk2 = nc.dram_tensor("bk2", [P, 2], F32, kind="ExternalInput")
    bvr = nc.dram_tensor("bvr", [1, GW], F32, kind="ExternalInput")
    out = nc.dram_tensor("out", [S, D_MODEL], F16, kind="ExternalOutput")

    consts = ctx.enter_context(tc.tile_pool(name="consts", bufs=1))
    persist = ctx.enter_context(tc.tile_pool(name="persist", bufs=1))
    xs = ctx.enter_context(tc.tile_pool(name="xs", bufs=3))
    xv_pool = ctx.enter_context(tc.tile_pool(name="xvs", bufs=3))
    sx = ctx.enter_context(tc.tile_pool(name="stexp", bufs=5))
    nrm = ctx.enter_context(tc.tile_pool(name="nrm", bufs=2))
    outp = ctx.enter_context(tc.tile_pool(name="outp", bufs=3))
    o1pool = ctx.enter_context(tc.tile_pool(name="o1pool", bufs=8))
    psA = ctx.enter_context(tc.tile_pool(name="psA", bufs=2, space="PSUM"))
    psB = ctx.enter_context(tc.tile_pool(name="psB", bufs=1, space="PSUM"))
    psP = ctx.enter_context(tc.tile_pool(name="psP", bufs=2, space="PSUM"))

    # ---- weights / constants (queue-balanced) ---------------------------
    wq_sb = consts.tile([P, 8, GW], F16)
    wk_sb = consts.tile([P, 8, GW], F16)
    wv_sb = consts.tile([P, 8, GW], F16)
    wo_sb = consts.tile([P, 2, D_MODEL], F16)
    bq_sb = consts.tile([P, 2], F32)
    bk_sb = consts.tile([P, 2], F32)
    bv_row = consts.tile([1, GW], F32)
    nc.sync.dma_start(wq_sb[:], wqT[:].rearrange("(c p) j -> p c j", p=P))
    nc.sync.dma_start(bq_sb[:], bq2[:])
    nc.scalar.dma_start(wk_sb[:], wkT[:].rearrange("(c p) j -> p c j", p=P))
    nc.scalar.dma_start(bk_sb[:], bk2[:])
    nc.gpsimd.dma_start(bv_row[:], bvr[:])
    bvb = consts.tile([P, GW], F32)
    nc.gpsimd.partition_broadcast(bvb[:], bv_row[:])

    # persistent activations.  QT[jc]: projected Q^T, partitions = the 128
    # features of chunk jc (head (jc,0) rows 0-63, head (jc,1) rows 64-127),
    # free dim = q.  QTw = partition-halves-swapped copy (for PE row-tiling).
    # After attention, OT overwrites QT's dead region (per-head rows).
    QT = [persist.tile([P, S], F16, name=f"QT{j}") for j in range(2)]
    QTw = [persist.tile([P, S], F16, name=f"QTw{j}") for j in range(2)]
    KT = [[persist.tile([P, 1024], F16, name=f"KT{j}_{sb}") for sb in range(2)]
          for j in range(2)]
    KTw = [[persist.tile([P, 1024], F16, name=f"KTw{j}_{sb}") for sb in range(2)]
           for j in range(2)]
    # V + ones column, per (kblock, head): lhsT of attn@V
    Vaug = persist.tile([P, 16, HPG, DK + 1], F16, name="Vaug")
    ones_f32 = consts.tile([P, 16, HPG], F32)
    nc.vector.memset(ones_f32[:], 1.0)
    # warm the ACT exp table during the DMA-bound prologue
    warm = consts.tile([1, 1], F32)
    nc.scalar.activation(out=warm[:], in_=ones_f32[0:1, 0, 0:1], func=EXPF)
    nc.vector.tensor_scalar_add(Vaug[:, :, :, DK], ones_f32[:], 0.0)

    xqT_r = xqT[:].rearrange("(c p) s -> p c s", p=P)
    xkT_r = xkT[:].rearrange("(c p) s -> p c s", p=P)
    xvT_r = xvT[:].rearrange("(c p) s -> p c s", p=P)

    # ---- projection work units ------------------------------------------
    def stage_x(x_r, sb, name, eng=None, split=2):
        """Stage one s-half of an input: 8 [128,1024] chunks in `split` DMAs."""
        eng = eng or nc.sync
        xt = xs.tile([P, 8, 1024], F16, tag="xs", name=f"x{name}{sb}")
        step = 8 // split
        for g in range(split):
            eng.dma_start(xt[:, g * step:(g + 1) * step, :],
                          x_r[:, g * step:(g + 1) * step,
                              sb * 1024:(sb + 1) * 1024])
        return xt

    def qk_mms(xt, w_sb, b_sb, dst, jc, sb, ns, full_s=False):
        """Projection matmuls + main drain for 512 s-columns of chunk jc.
        Returns a closure writing the partition-swapped copy (reads the same
        PSUM tile, so it must run before psP recycles this buffer)."""
        ps = psP.tile([P, 512], F32, tag="psP", name=f"ps{jc}{sb}{ns}")
        for i in range(8):
            nc.tensor.matmul(
                ps[:, :],
                w_sb[:, i, jc * P:(jc + 1) * P],
                xt[:, i, ns * 512:(ns + 1) * 512],
                start=(i == 0), stop=(i == 7),
            )
        c0 = (sb * 1024 if full_s else 0) + ns * 512
        sl = slice(c0, c0 + 512)
        nc.vector.tensor_scalar_add(dst[:, sl], ps[:], b_sb[:, jc:jc + 1])

        def swaps(dstw):
            nc.vector.tensor_scalar_add(dstw[64:128, sl], ps[0:64, :],
                                        b_sb[0:64, jc:jc + 1])
            nc.vector.tensor_scalar_add(dstw[0:64, sl], ps[64:128, :],
                                        b_sb[64:128, jc:jc + 1])
        return swaps

    def qk_unit(xt, w_sb, b_sb, dst, dstw, jc, sb, ns, full_s=False):
        qk_mms(xt, w_sb, b_sb, dst, jc, sb, ns, full_s)(dstw)

    def v_unit(kb):
        """V projection for kblock kb -> Vaug[:, kb, :, 0:64] (+bias)."""
        xvt = xv_pool.tile([P, 8, P], F16, tag="xv", name=f"xv{kb}")
        nc.gpsimd.dma_start(xvt[:], xvT_r[:, :, kb * P:(kb + 1) * P])
        pv = psP.tile([P, 512], F32, tag="psP", name=f"pv{kb}")
        for i in range(8):
            nc.tensor.matmul(
                pv[:, 0:GW], xvt[:, i, :], wv_sb[:, i, :],
                start=(i == 0), stop=(i == 7),
            )
        nc.vector.tensor_tensor(
            Vaug[:, kb, :, 0:DK],
            pv[:, 0:GW].rearrange("p (h d) -> p h d", h=HPG),
            bvb[:].rearrange("p (h d) -> p h d", h=HPG),
            ADD,
        )

    ot_tiles = {}

    def oproj_full(sc):
        """Complete O projection for s-chunk sc (both halves + store)."""
        ot = outp.tile([P, 1024], F16, tag="osb", name=f"ot{sc}")
        for ms in range(2):
            pso = psP.tile([P, 512], F32, tag="psP", name=f"pso{sc}{ms}")
            for hd in range(2):
                nc.tensor.matmul(
                    pso[:, :],
                    QT[hd][:, sc * P:(sc + 1) * P],
                    wo_sb[:, hd, ms * 512:(ms + 1) * 512],
                    start=(hd == 0), stop=(hd == 1),
                )
            nc.vector.tensor_copy(out=ot[:, ms * 512:(ms + 1) * 512],
                                  in_=pso[:, :])
        eng = nc.sync if sc % 2 == 0 else nc.gpsimd
        eng.dma_start(out[sc * P:(sc + 1) * P, :], ot[:])

    def oproj_part1(sc):
        """hd=0 half of the contraction for s-chunk sc -> SBUF staging."""
        ot = o1pool.tile([P, 1024], F32, tag="o1", name=f"o1_{sc}")
        ot_tiles[sc] = ot
        for ms in range(2):
            pso = psP.tile([P, 512], F32, tag="psP", name=f"p1_{sc}{ms}")
            nc.tensor.matmul(pso[:, :], QT[0][:, sc * P:(sc + 1) * P],
                             wo_sb[:, 0, ms * 512:(ms + 1) * 512],
                             start=True, stop=True)
            nc.vector.tensor_copy(out=ot[:, ms * 512:(ms + 1) * 512],
                                  in_=pso[:, :])

    def oproj_part2(sc):
        """hd=1 half + add + store for s-chunk sc."""
        ot1 = ot_tiles[sc]
        ot2 = outp.tile([P, 1024], F16, tag="osb2", name=f"o2_{sc}")
        for ms in range(2):
            pso = psP.tile([P, 512], F32, tag="psP", name=f"p2_{sc}{ms}")
            nc.tensor.matmul(pso[:, :], QT[1][:, sc * P:(sc + 1) * P],
                             wo_sb[:, 1, ms * 512:(ms + 1) * 512],
                             start=True, stop=True)
            nc.vector.tensor_tensor(ot2[:, ms * 512:(ms + 1) * 512], pso[:, :],
                                    ot1[:, ms * 512:(ms + 1) * 512], ADD)
        eng = nc.sync if sc % 2 == 0 else nc.gpsimd
        eng.dma_start(out[sc * P:(sc + 1) * P, :], ot2[:])

    # ---- attention pieces ----------------------------------------------
    _pst_box = [None]

    def scores_pair(h, qb, kb):
        """ST[k,q]: two concurrent 64-row-tile matmuls (home + swapped)."""
        jc, hp = h // 2, h % 2
        pr = 64 * hp
        prw = 64 - pr
        sb, kc = kb // 8, (kb % 8) * P
        pst = psA.tile([P, 1024], F32, tag="psA", name=f"pst{h}{qb}{kb}")
        _pst_box[0] = pst
        q0 = qb * 1024
        nc.tensor.matmul(
            pst[:, 0:512],
            KT[jc][sb][pr:pr + DK, kc:kc + P],
            QT[jc][pr:pr + DK, q0:q0 + 512],
            start=True, stop=True,
        )
        nc.tensor.matmul(
            pst[:, 512:1024],
            KTw[jc][sb][prw:prw + DK, kc:kc + P],
            QTw[jc][prw:prw + DK, q0 + 512:q0 + 1024],
            start=True, stop=True,
        )

    def attn_v(po, st, h, kb):
        for ns in range(2):
            nc.tensor.matmul(
                po[0:DK + 1, ns * 512:(ns + 1) * 512],
                Vaug[:, kb, h, :],
                st[:, ns * 512:(ns + 1) * 512],
                start=(kb == 0), stop=(kb == 15),
            )

    def normalize(po, h, qb):
        """Copy po out of PSUM (frees the single po buffer), then scale by
        the reciprocal of the denominators (row DK) and write OT into QT."""
        jc, hp = h // 2, h % 2
        pr = 64 * hp
        poc = nrm.tile([DK + 1, 1024], F32, tag="poc", name=f"poc{h}{qb}")
        dn = nrm.tile([1, 1024], F32, tag="dn", name=f"dn{h}{qb}")
        nc.vector.tensor_copy(out=poc[:], in_=po[0:DK + 1, :])
        nc.vector.tensor_copy(out=dn[:], in_=poc[DK:DK + 1, :])
        bc = nrm.tile([DK, 1024], F32, tag="bcast", name=f"bc{h}{qb}")
        nc.vector.reciprocal_approx_fast(bc[0:1, :], dn[:])
        nc.gpsimd.partition_broadcast(bc[:], bc[0:1, :])
        q0 = qb * 1024
        nc.vector.tensor_tensor(
            QT[jc][pr:pr + DK, q0:q0 + 1024], poc[0:DK, :], bc[:], MULT)

    # ---- prologue -------------------------------------------------------
    # xq0 on sync, xk0 on scalar (idle until the first exp), weights split.
    xq0 = stage_x(xqT_r, 0, "q", eng=nc.sync, split=2)
    xk0 = stage_x(xkT_r, 0, "k", eng=nc.scalar, split=2)
    dum = consts.tile([P, 512], F16)
    nc.vector.memset(dum[:], 0.001)
    warm_ps = psA.tile([P, 1024], F32, tag="psA", name="warmps")
    for _ in range(14):
        nc.tensor.matmul(warm_ps[:, 0:512], dum[:, 0:128], dum[:, :],
                         start=True, stop=True)
    sw_q0 = qk_mms(xq0, wq_sb, bq_sb, QT[0], 0, 0, 0, full_s=True)
    sw_q1 = qk_mms(xq0, wq_sb, bq_sb, QT[0], 0, 0, 1, full_s=True)
    sw_q0(QTw[0])
    sw_q1(QTw[0])
    sw_k0 = qk_mms(xk0, wk_sb, bk_sb, KT[0][0], 0, 0, 0)
    sw_k0(KTw[0][0])
    sw_k1 = qk_mms(xk0, wk_sb, bk_sb, KT[0][0], 0, 0, 1)
    sw_k1(KTw[0][0])
    wvT_r = wvT[:].rearrange("(c p) j -> p c j", p=P)
    nc.scalar.dma_start(wv_sb[:], wvT_r)
    woT_r = woT[:].rearrange("(c p) m -> p c m", p=P)
    for cc in range(2):
        nc.sync.dma_start(wo_sb[:, cc, :], woT_r[:, cc, :])
    for kb in range(4):
        v_unit(kb)
    xk1 = stage_x(xkT_r, 1, "k1", eng=nc.scalar, split=2)

    # deferred projection / oproj work, as (slot -> closure) over the stream
    box = {}
    fills = {}

    def put(step, fn):
        fills.setdefault(step, []).append(fn)

    # u0 (h0,qb0): remaining V units inline + the sb1 K units it needs
    for kb in range(12):
        put(kb, lambda kb=kb: v_unit(kb + 4))
    put(1, lambda: qk_unit(xk1, wk_sb, bk_sb, KT[0][1], KTw[0][1], 0, 1, 0))
    put(4, lambda: qk_unit(xk1, wk_sb, bk_sb, KT[0][1], KTw[0][1], 0, 1, 1))
    # u1 (h1,qb0): Q/K for the jc=1 heads + stage xq sb1
    put(17, lambda: qk_unit(xq0, wq_sb, bq_sb, QT[1], QTw[1], 1, 0, 0,
                            full_s=True))
    put(19, lambda: qk_unit(xq0, wq_sb, bq_sb, QT[1], QTw[1], 1, 0, 1,
                            full_s=True))
    put(21, lambda: qk_unit(xk0, wk_sb, bk_sb, KT[1][0], KTw[1][0], 1, 0, 0))
    put(23, lambda: qk_unit(xk0, wk_sb, bk_sb, KT[1][0], KTw[1][0], 1, 0, 1))
    put(26, lambda: box.__setitem__(
        "xq1", stage_x(xqT_r, 1, "q1", eng=nc.gpsimd, split=2)))
    # u2 (h2,qb0): jc=1 sb1 K units + Q sb1 units
    put(33, lambda: qk_unit(xk1, wk_sb, bk_sb, KT[1][1], KTw[1][1], 1, 1, 0))
    put(35, lambda: qk_unit(xk1, wk_sb, bk_sb, KT[1][1], KTw[1][1], 1, 1, 1))
    put(38, lambda: qk_unit(box["xq1"], wq_sb, bq_sb, QT[0], QTw[0], 0, 1, 0,
                            full_s=True))
    put(41, lambda: qk_unit(box["xq1"], wq_sb, bq_sb, QT[0], QTw[0], 0, 1, 1,
                            full_s=True))
    put(44, lambda: qk_unit(box["xq1"], wq_sb, bq_sb, QT[1], QTw[1], 1, 1, 0,
                            full_s=True))
    put(47, lambda: qk_unit(box["xq1"], wq_sb, bq_sb, QT[1], QTw[1], 1, 1, 1,
                            full_s=True))
    # qb0 O-projection during u4,u5 (needs all qb0 heads normalized)
    for j, sc in enumerate(range(8)):
        put(68 + 3 * j, lambda sc=sc: oproj_full(sc))
    # qb1 O-projection: hd0 half during u6/u7, hd1 half in the tail
    for j, sc in enumerate(range(8, 16)):
        put(99 + 3 * j, lambda sc=sc: oproj_part1(sc))

    # ---- the flat attention stream -------------------------------------
    units = [(h, qb) for qb in (0, 1) for h in range(4)]
    seq = [(u, kb) for u in range(8) for kb in range(16)]
    po_of = {}
    sts = {}

    def do_lagged(i):
        u2, kb2 = seq[i]
        h2, qb2 = units[u2]
        if kb2 == 0:
            po_of[u2] = psB.tile([P, 1024], F32, tag="po", name=f"po{u2}")
        attn_v(po_of[u2], sts[(u2, kb2)], h2, kb2)
        del sts[(u2, kb2)]
        if kb2 == 15:
            normalize(po_of[u2], h2, qb2)

    for i, (u, kb) in enumerate(seq):
        h, qb = units[u]
        scores_pair(h, qb, kb)
        st = sx.tile([P, 1024], F16, tag="st", name=f"st{u}_{kb}")
        nc.scalar.activation(out=st[:], in_=_pst_box[0][:], func=EXPF,
                             scale=0.125)
        sts[(u, kb)] = st
        if i >= LAG:
            do_lagged(i - LAG)
        for fn in fills.pop(i, ()):
            fn()
    for i in range(len(seq) - LAG, len(seq)):
        do_lagged(i)
    # tail: finish qb1 O-projection
    for sc in range(8, 16):
        oproj_part2(sc)


_prog_cache = {}


def _build_program():
    if "nc" not in _prog_cache:
        from contextlib import ExitStack
        nc = bacc.Bacc("TRN2", target_bir_lowering=False)
        with tile.TileContext(nc) as tc:
            with ExitStack() as ctx:
                _emit(nc, tc, ctx)
        nc.compile()
        _prog_cache["nc"] = nc
    return _prog_cache["nc"]


def make_in_maps(query, key, value, Wq, bq, Wk, bk, Wv, bv, Wo, bo):
    query, key, value = (np.asarray(t, np.float32) for t in (query, key, value))
    Wq, Wk, Wv, Wo = (np.asarray(t, np.float32) for t in (Wq, Wk, Wv, Wo))
    bq, bk, bv = (np.asarray(t, np.float32) for t in (bq, bk, bv))
    xT = {b: {} for b in range(B)}
    for b in range(B):
        xT[b]["q"] = np.ascontiguousarray(query[b].T).astype(np.float16)
        xT[b]["k"] = np.ascontiguousarray(key[b].T).astype(np.float16)
        xT[b]["v"] = np.ascontiguousarray(value[b].T).astype(np.float16)
    in_maps = []
    for c in range(N_CORES):
        b, g = divmod(c, GROUPS)
        gs = slice(g * GW, (g + 1) * GW)
        in_maps.append({
            "xqT": xT[b]["q"], "xkT": xT[b]["k"], "xvT": xT[b]["v"],
            "wqT": np.ascontiguousarray(Wq[gs, :].T).astype(np.float16),
            "wkT": np.ascontiguousarray(Wk[gs, :].T).astype(np.float16),
            "wvT": np.ascontiguousarray(Wv[gs, :].T).astype(np.float16),
            "woT": np.ascontiguousarray(Wo[:, gs].T).astype(np.float16),
            "bq2": np.ascontiguousarray(bq[gs].reshape(2, 128).T),
            "bk2": np.ascontiguousarray(bk[gs].reshape(2, 128).T),
            "bvr": np.ascontiguousarray(bv[gs].reshape(1, GW)),
        })
    return in_maps


def run_on_hw(in_maps, trace=False, **kw):
    nc = _build_program()
    return run_bass_kernel_spmd(nc, in_maps, core_ids=list(range(N_CORES)),
                                trace=trace, **kw)


def kernel(query, key, value, Wq, bq, Wk, bk, Wv, bv, Wo, bo):
    in_maps = make_in_maps(query, key, value, Wq, bq, Wk, bk, Wv, bv, Wo, bo)
    res = run_on_hw(in_maps)
    out = np.zeros((B, S, D_MODEL), np.float32)
    for c in range(N_CORES):
        out[c // GROUPS] += res.results[c]["out"].astype(np.float32)
    out += np.asarray(bo, np.float32)
    return out



# revision 42
# speedup vs baseline: 1.0111x; 1.0105x over previous
"""Multi-head attention (B=2, S=2048, D=1024, H=16) on 8 Trainium2 NeuronCores.

Sharding: tensor-parallel on heads (4 groups of 4 heads) x data-parallel on
batch (2) -> 8 cores. Each core computes QKV projections for its head slice,
attention for its 4 heads, and a partial output projection; the host sums the
4 partials per batch element (the tensor-parallel allreduce) and adds bo.

v3 design: the softmax exp stream on the ACT engine is the pacing resource
(~135us); everything else hides under it.
- One flat stream of (head, q-block, kblock) steps: scores -> exp each step,
  attn@V lagging 3 steps behind, so head boundaries never stall the exp
  stream (the in-order PE queue always has ready work in front).
- Scores are computed transposed (ST[k,q] = K_h.T @ Q_h) per (head, kblock)
  into a 2-buffer x [128,1024] PSUM ring (4 banks) -> exp runs at FD=1024
  with no ping-pong stalls.  The two 512-wide score matmuls of each step run
  CONCURRENTLY on disjoint 64-row PE tiles (K/Q live in both partition
  halves via a partition-swapped copy), so dk=64 wastes nothing.
- attn@V accumulates po[65, 1024] in PSUM (2 banks); a ones-column appended
  to V makes row 64 the softmax denominators.  Normalize first copies po to
  SBUF (freeing the single po buffer early), then recip/broadcast/scale.
- QKV/O projections are [128,512]-output units on a 2-buffer PSUM pool
  (2 banks), interleaved into the stream to fill PE slack.  Prologue DMAs
  are spread across the sync/scalar/gpsimd queues (the ~625ns per-DMA issue
  cost on one queue would otherwise serialize the start).
"""

import numpy as np

import concourse.bass as bass  # noqa: F401
import concourse.tile as tile
from concourse import bacc, mybir
from concourse.bass_utils import run_bass_kernel_spmd

D_MODEL = 1024
NUM_HEADS = 16
DK = 64
B, S = 2, 2048
N_CORES = 8
GROUPS = 4                 # head groups (tensor parallel)
GW = D_MODEL // GROUPS     # 256 features per group = 4 heads
HPG = GROUPS               # heads per group = 4

F32 = mybir.dt.float32
F16 = mybir.dt.float16
EXPF = mybir.ActivationFunctionType.Exp
MULT = mybir.AluOpType.mult
ADD = mybir.AluOpType.add

LAG = 3                    # attn@V trails exp by this many stream steps


def _emit(nc, tc, ctx):
    P = 128
    xqT = nc.dram_tensor("xqT", [D_MODEL, S], F16, kind="ExternalInput")
    xkT = nc.dram_tensor("xkT", [D_MODEL, S], F16, kind="ExternalInput")
    xvT = nc.dram_tensor("xvT", [D_MODEL, S], F16, kind="ExternalInput")
    wqT = nc.dram_tensor("wqT", [D_MODEL, GW], F16, kind="ExternalInput")
    wkT = nc.dram_tensor("wkT", [D_MODEL, GW], F16, kind="ExternalInput")
    wvT = nc.dram_tensor("wvT", [D_MODEL, GW], F16, kind="ExternalInput")
    woT = nc.dram_tensor("woT", [GW, D_MODEL], F16, kind="ExternalInput")
    bq2 = nc.dram_tensor("bq2", [P, 2], F32, kind="ExternalInput")
    bk2 = nc.dram_tensor("bk2", [P, 2], F32, kind="ExternalInput")
    bvr = nc.dram_tensor("bvr", [1, GW], F32, kind="ExternalInput")
    out = nc.dram_tensor("out", [S, D_MODEL], F16, kind="ExternalOutput")

    consts = ctx.enter_context(tc.tile_pool(name="consts", bufs=1))
    persist = ctx.enter_context(tc.tile_pool(name="persist", bufs=1))
    xs = ctx.enter_context(tc.tile_pool(name="xs", bufs=3))
    xv_pool = ctx.enter_context(tc.tile_pool(name="xvs", bufs=3))
    sx = ctx.enter_context(tc.tile_pool(name="stexp", bufs=5))
    nrm = ctx.enter_context(tc.tile_pool(name="nrm", bufs=2))
    outp = ctx.enter_context(tc.tile_pool(name="outp", bufs=3))
    o1pool = ctx.enter_context(tc.tile_pool(name="o1pool", bufs=8))
    psA = ctx.enter_context(tc.tile_pool(name="psA", bufs=2, space="PSUM"))
    psB = ctx.enter_context(tc.tile_pool(name="psB", bufs=1, space="PSUM"))
    psP = ctx.enter_context(tc.tile_pool(name="psP", bufs=2, space="PSUM"))

    # ---- weights / constants (queue-balanced) ---------------------------
    wq_sb = consts.tile([P, 8, GW], F16)
    wk_sb = consts.tile([P, 8, GW], F16)
    wv_sb = consts.tile([P, 8, GW], F16)
    wo_sb = consts.tile([P, 2, D_MODEL], F16)
    bq_sb = consts.tile([P, 2], F32)
    bk_sb = consts.tile([P, 2], F32)
    bv_row = consts.tile([1, GW], F32)
    nc.sync.dma_start(wq_sb[:], wqT[:].rearrange("(c p) j -> p c j", p=P))
    nc.sync.dma_start(bq_sb[:], bq2[:])
    nc.scalar.dma_start(wk_sb[:], wkT[:].rearrange("(c p) j -> p c j", p=P))
    nc.scalar.dma_start(bk_sb[:], bk2[:])
    nc.gpsimd.dma_start(bv_row[:], bvr[:])
    bvb = consts.tile([P, GW], F32)
    nc.gpsimd.partition_broadcast(bvb[:], bv_row[:])

    # persistent activations.  QT[jc]: projected Q^T, partitions = the 128
    # features of chunk jc (head (jc,0) rows 0-63, head (jc,1) rows 64-127),
    # free dim = q.  QTw = partition-halves-swapped copy (for PE row-tiling).
    # After attention, OT overwrites QT's dead region (per-head rows).
    QT = [persist.tile([P, S], F16, name=f"QT{j}") for j in range(2)]
    QTw = [persist.tile([P, S], F16, name=f"QTw{j}") for j in range(2)]
    KT = [[persist.tile([P, 1024], F16, name=f"KT{j}_{sb}") for sb in range(2)]
          for j in range(2)]
    KTw = [[persist.tile([P, 1024], F16, name=f"KTw{j}_{sb}") for sb in range(2)]
           for j in range(2)]
    # V + ones column, per (kblock, head): lhsT of attn@V
    Vaug = persist.tile([P, 16, HPG, DK + 1], F16, name="Vaug")
    ones_f32 = consts.tile([P, 16, HPG], F32)
    nc.vector.memset(ones_f32[:], 1.0)
    # warm the ACT exp table during the DMA-bound prologue
    warm = consts.tile([1, 1], F32)
    nc.scalar.activation(out=warm[:], in_=ones_f32[0:1, 0, 0:1], func=EXPF)
    nc.vector.tensor_scalar_add(Vaug[:, :, :, DK], ones_f32[:], 0.0)

    xqT_r = xqT[:].rearrange("(c p) s -> p c s", p=P)
    xkT_r = xkT[:].rearrange("(c p) s -> p c s", p=P)
    xvT_r = xvT[:].rearrange("(c p) s -> p c s", p=P)

    # ---- projection work units ------------------------------------------
    def stage_x(x_r, sb, name, eng=None, split=2):
        """Stage one s-half of an input: 8 [128,1024] chunks in `split` DMAs."""
        eng = eng or nc.sync
        xt = xs.tile([P, 8, 1024], F16, tag="xs", name=f"x{name}{sb}")
        step = 8 // split
        for g in range(split):
            eng.dma_start(xt[:, g * step:(g + 1) * step, :],
                          x_r[:, g * step:(g + 1) * step,
                              sb * 1024:(sb + 1) * 1024])
        return xt

    def qk_mms(xt, w_sb, b_sb, dst, jc, sb, ns, full_s=False):
        """Projection matmuls + main drain for 512 s-columns of chunk jc.
        Returns a closure writing the partition-swapped copy (reads the same
        PSUM tile, so it must run before psP recycles this buffer)."""
        ps = psP.tile([P, 512], F32, tag="psP", name=f"ps{jc}{sb}{ns}")
        for i in range(8):
            nc.tensor.matmul(
                ps[:, :],
                w_sb[:, i, jc * P:(jc + 1) * P],
                xt[:, i, ns * 512:(ns + 1) * 512],
                start=(i == 0), stop=(i == 7),
            )
        c0 = (sb * 1024 if full_s else 0) + ns * 512
        sl = slice(c0, c0 + 512)
        nc.vector.tensor_scalar_add(dst[:, sl], ps[:], b_sb[:, jc:jc + 1])

        def swaps(dstw):
            nc.vector.tensor_scalar_add(dstw[64:128, sl], ps[0:64, :],
                                        b_sb[0:64, jc:jc + 1])
            nc.vector.tensor_scalar_add(dstw[0:64, sl], ps[64:128, :],
                                        b_sb[64:128, jc:jc + 1])
        return swaps

    def qk_unit(xt, w_sb, b_sb, dst, dstw, jc, sb, ns, full_s=False):
        qk_mms(xt, w_sb, b_sb, dst, jc, sb, ns, full_s)(dstw)

    def v_unit(kb, staged=None):
        """V projection for kblock kb -> Vaug[:, kb, :, 0:64] (+bias)."""
        if staged is None:
            xvt = xv_pool.tile([P, 8, P], F16, tag="xv", name=f"xv{kb}")
            nc.gpsimd.dma_start(xvt[:], xvT_r[:, :, kb * P:(kb + 1) * P])
            src_ap = xvt
            c0 = 0
        else:
            src_ap = staged
            c0 = (kb % 4) * P
        pv = psP.tile([P, 512], F32, tag="psP", name=f"pv{kb}")
        for i in range(8):
            nc.tensor.matmul(
                pv[:, 0:GW], src_ap[:, i, c0:c0 + P], wv_sb[:, i, :],
                start=(i == 0), stop=(i == 7),
            )
        nc.vector.tensor_tensor(
            Vaug[:, kb, :, 0:DK],
            pv[:, 0:GW].rearrange("p (h d) -> p h d", h=HPG),
            bvb[:].rearrange("p (h d) -> p h d", h=HPG),
            ADD,
        )

    ot_tiles = {}

    def oproj_full(sc):
        """Complete O projection for s-chunk sc (both halves + store)."""
        ot = outp.tile([P, 1024], F16, tag="osb", name=f"ot{sc}")
        for ms in range(2):
            pso = psP.tile([P, 512], F32, tag="psP", name=f"pso{sc}{ms}")
            for hd in range(2):
                nc.tensor.matmul(
                    pso[:, :],
                    QT[hd][:, sc * P:(sc + 1) * P],
                    wo_sb[:, hd, ms * 512:(ms + 1) * 512],
                    start=(hd == 0), stop=(hd == 1),
                )
            nc.vector.tensor_copy(out=ot[:, ms * 512:(ms + 1) * 512],
                                  in_=pso[:, :])
        eng = nc.sync if sc % 2 == 0 else nc.gpsimd
        eng.dma_start(out[sc * P:(sc + 1) * P, :], ot[:])

    def oproj_part1(sc):
        """hd=0 half of the contraction for s-chunk sc -> SBUF staging."""
        ot = o1pool.tile([P, 1024], F32, tag="o1", name=f"o1_{sc}")
        ot_tiles[sc] = ot
        for ms in range(2):
            pso = psP.tile([P, 512], F32, tag="psP", name=f"p1_{sc}{ms}")
            nc.tensor.matmul(pso[:, :], QT[0][:, sc * P:(sc + 1) * P],
                             wo_sb[:, 0, ms * 512:(ms + 1) * 512],
                             start=True, stop=True)
            nc.vector.tensor_copy(out=ot[:, ms * 512:(ms + 1) * 512],
                                  in_=pso[:, :])

    def oproj_part2(sc):
        """hd=1 half + add + store for s-chunk sc."""
        ot1 = ot_tiles[sc]
        ot2 = outp.tile([P, 1024], F16, tag="osb2", name=f"o2_{sc}")
        for ms in range(2):
            pso = psP.tile([P, 512], F32, tag="psP", name=f"p2_{sc}{ms}")
            nc.tensor.matmul(pso[:, :], QT[1][:, sc * P:(sc + 1) * P],
                             wo_sb[:, 1, ms * 512:(ms + 1) * 512],
                             start=True, stop=True)
            nc.vector.tensor_tensor(ot2[:, ms * 512:(ms + 1) * 512], pso[:, :],
                                    ot1[:, ms * 512:(ms + 1) * 512], ADD)
        eng = nc.sync if sc % 2 == 0 else nc.gpsimd
        eng.dma_start(out[sc * P:(sc + 1) * P, :], ot2[:])

    # ---- attention pieces ----------------------------------------------
    _pst_box = [None]

    def scores_pair(h, qb, kb):
        """ST[k,q]: two concurrent 64-row-tile matmuls (home + swapped)."""
        jc, hp = h // 2, h % 2
        pr = 64 * hp
        prw = 64 - pr
        sb, kc = kb // 8, (kb % 8) * P
        pst = psA.tile([P, 1024], F32, tag="psA", name=f"pst{h}{qb}{kb}")
        _pst_box[0] = pst
        q0 = qb * 1024
        nc.tensor.matmul(
            pst[:, 0:512],
            KT[jc][sb][pr:pr + DK, kc:kc + P],
            QT[jc][pr:pr + DK, q0:q0 + 512],
            start=True, stop=True,
        )
        nc.tensor.matmul(
            pst[:, 512:1024],
            KTw[jc][sb][prw:prw + DK, kc:kc + P],
            QTw[jc][prw:prw + DK, q0 + 512:q0 + 1024],
            start=True, stop=True,
        )

    def attn_v(po, st, h, kb):
        for ns in range(2):
            nc.tensor.matmul(
                po[0:DK + 1, ns * 512:(ns + 1) * 512],
                Vaug[:, kb, h, :],
                st[:, ns * 512:(ns + 1) * 512],
                start=(kb == 0), stop=(kb == 15),
            )

    def normalize(po, h, qb):
        """Copy po out of PSUM (frees the single po buffer), then scale by
        the reciprocal of the denominators (row DK) and write OT into QT."""
        jc, hp = h // 2, h % 2
        pr = 64 * hp
        poc = nrm.tile([DK + 1, 1024], F32, tag="poc", name=f"poc{h}{qb}")
        dn = nrm.tile([1, 1024], F32, tag="dn", name=f"dn{h}{qb}")
        nc.vector.tensor_copy(out=poc[:], in_=po[0:DK + 1, :])
        nc.vector.tensor_copy(out=dn[:], in_=poc[DK:DK + 1, :])
        bc = nrm.tile([DK, 1024], F32, tag="bcast", name=f"bc{h}{qb}")
        nc.vector.reciprocal_approx_fast(bc[0:1, :], dn[:])
        nc.gpsimd.partition_broadcast(bc[:], bc[0:1, :])
        q0 = qb * 1024
        nc.vector.tensor_tensor(
            QT[jc][pr:pr + DK, q0:q0 + 1024], poc[0:DK, :], bc[:], MULT)

    # ---- prologue -------------------------------------------------------
    # xq0 on sync, xk0 on scalar (idle until the first exp), weights split.
    xq0 = stage_x(xqT_r, 0, "q", eng=nc.sync, split=2)
    xk0 = stage_x(xkT_r, 0, "k", eng=nc.scalar, split=2)
    dum = consts.tile([P, 512], F16)
    nc.vector.memset(dum[:], 0.001)
    warm_ps = psA.tile([P, 1024], F32, tag="psA", name="warmps")
    for _ in range(14):
        nc.tensor.matmul(warm_ps[:, 0:512], dum[:, 0:128], dum[:, :],
                         start=True, stop=True)
    sw_q0 = qk_mms(xq0, wq_sb, bq_sb, QT[0], 0, 0, 0, full_s=True)
    sw_q1 = qk_mms(xq0, wq_sb, bq_sb, QT[0], 0, 0, 1, full_s=True)
    sw_q0(QTw[0])
    sw_q1(QTw[0])
    sw_k0 = qk_mms(xk0, wk_sb, bk_sb, KT[0][0], 0, 0, 0)
    sw_k0(KTw[0][0])
    sw_k1 = qk_mms(xk0, wk_sb, bk_sb, KT[0][0], 0, 0, 1)
    sw_k1(KTw[0][0])
    wvT_r = wvT[:].rearrange("(c p) j -> p c j", p=P)
    nc.gpsimd.dma_start(wv_sb[:], wvT_r)
    xv03 = consts.tile([P, 8, 512], F16)
    nc.sync.dma_start(xv03[:], xvT_r[:, :, 0:512])
    woT_r = woT[:].rearrange("(c p) m -> p c m", p=P)
    for cc in range(2):
        nc.sync.dma_start(wo_sb[:, cc, :], woT_r[:, cc, :])
    for kb in range(4):
        v_unit(kb, xv03)
    xk1 = stage_x(xkT_r, 1, "k1", eng=nc.scalar, split=2)

    # deferred projection / oproj work, as (slot -> closure) over the stream
    box = {}
    fills = {}

    def put(step, fn):
        fills.setdefault(step, []).append(fn)

    # u0 (h0,qb0): remaining V units inline + the sb1 K units it needs
    for kb in range(12):
        put(kb, lambda kb=kb: v_unit(kb + 4))
    put(1, lambda: qk_unit(xk1, wk_sb, bk_sb, KT[0][1], KTw[0][1], 0, 1, 0))
    put(4, lambda: qk_unit(xk1, wk_sb, bk_sb, KT[0][1], KTw[0][1], 0, 1, 1))
    # u1 (h1,qb0): Q/K for the jc=1 heads + stage xq sb1
    put(17, lambda: qk_unit(xq0, wq_sb, bq_sb, QT[1], QTw[1], 1, 0, 0,
                            full_s=True))
    put(19, lambda: qk_unit(xq0, wq_sb, bq_sb, QT[1], QTw[1], 1, 0, 1,
                            full_s=True))
    put(21, lambda: qk_unit(xk0, wk_sb, bk_sb, KT[1][0], KTw[1][0], 1, 0, 0))
    put(23, lambda: qk_unit(xk0, wk_sb, bk_sb, KT[1][0], KTw[1][0], 1, 0, 1))
    put(26, lambda: box.__setitem__(
        "xq1", stage_x(xqT_r, 1, "q1", eng=nc.gpsimd, split=2)))
    # u2 (h2,qb0): jc=1 sb1 K units + Q sb1 units
    put(33, lambda: qk_unit(xk1, wk_sb, bk_sb, KT[1][1], KTw[1][1], 1, 1, 0))
    put(35, lambda: qk_unit(xk1, wk_sb, bk_sb, KT[1][1], KTw[1][1], 1, 1, 1))
    put(38, lambda: qk_unit(box["xq1"], wq_sb, bq_sb, QT[0], QTw[0], 0, 1, 0,
                            full_s=True))
    put(41, lambda: qk_unit(box["xq1"], wq_sb, bq_sb, QT[0], QTw[0], 0, 1, 1,
                            full_s=True))
    put(44, lambda: qk_unit(box["xq1"], wq_sb, bq_sb, QT[1], QTw[1], 1, 1, 0,
                            full_s=True))
    put(47, lambda: qk_unit(box["xq1"], wq_sb, bq_sb, QT[1], QTw[1], 1, 1, 1,
                            full_s=True))
    # qb0 O-projection during u4,u5 (needs all qb0 heads normalized)
    for j, sc in enumerate(range(8)):
        put(68 + 3 * j, lambda sc=sc: oproj_full(sc))
    # qb1 O-projection: hd0 half during u6/u7, hd1 half in the tail
    for j, sc in enumerate(range(8, 16)):
        put(99 + 3 * j, lambda sc=sc: oproj_part1(sc))

    # ---- the flat attention stream -------------------------------------
    units = [(h, qb) for qb in (0, 1) for h in range(4)]
    seq = [(u, kb) for u in range(8) for kb in range(16)]
    po_of = {}
    sts = {}

    def do_lagged(i):
        u2, kb2 = seq[i]
        h2, qb2 = units[u2]
        if kb2 == 0:
            po_of[u2] = psB.tile([P, 1024], F32, tag="po", name=f"po{u2}")
        attn_v(po_of[u2], sts[(u2, kb2)], h2, kb2)
        del sts[(u2, kb2)]
        if kb2 == 15:
            normalize(po_of[u2], h2, qb2)

    for i, (u, kb) in enumerate(seq):
        h, qb = units[u]
        scores_pair(h, qb, kb)
        st = sx.tile([P, 1024], F16, tag="st", name=f"st{u}_{kb}")
        nc.scalar.activation(out=st[:], in_=_pst_box[0][:], func=EXPF,
                             scale=0.125)
        sts[(u, kb)] = st
        if i >= LAG:
            do_lagged(i - LAG)
        for fn in fills.pop(i, ()):
            fn()
    for i in range(len(seq) - LAG, len(seq)):
        do_lagged(i)
    # tail: finish qb1 O-projection
    for sc in range(8, 16):
        oproj_part2(sc)


_prog_cache = {}


def _build_program():
    if "nc" not in _prog_cache:
        from contextlib import ExitStack
        nc = bacc.Bacc("TRN2", target_bir_lowering=False)
        with tile.TileContext(nc) as tc:
            with ExitStack() as ctx:
                _emit(nc, tc, ctx)
        nc.compile()
        _prog_cache["nc"] = nc
    return _prog_cache["nc"]


def make_in_maps(query, key, value, Wq, bq, Wk, bk, Wv, bv, Wo, bo):
    query, key, value = (np.asarray(t, np.float32) for t in (query, key, value))
    Wq, Wk, Wv, Wo = (np.asarray(t, np.float32) for t in (Wq, Wk, Wv, Wo))
    bq, bk, bv = (np.asarray(t, np.float32) for t in (bq, bk, bv))
    xT = {b: {} for b in range(B)}
    for b in range(B):
        xT[b]["q"] = np.ascontiguousarray(query[b].T).astype(np.float16)
        xT[b]["k"] = np.ascontiguousarray(key[b].T).astype(np.float16)
        xT[b]["v"] = np.ascontiguousarray(value[b].T).astype(np.float16)
    in_maps = []
    for c in range(N_CORES):
        b, g = divmod(c, GROUPS)
        gs = slice(g * GW, (g + 1) * GW)
        in_maps.append({
            "xqT": xT[b]["q"], "xkT": xT[b]["k"], "xvT": xT[b]["v"],
            "wqT": np.ascontiguousarray(Wq[gs, :].T).astype(np.float16),
            "wkT": np.ascontiguousarray(Wk[gs, :].T).astype(np.float16),
            "wvT": np.ascontiguousarray(Wv[gs, :].T).astype(np.float16),
            "woT": np.ascontiguousarray(Wo[:, gs].T).astype(np.float16),
            "bq2": np.ascontiguousarray(bq[gs].reshape(2, 128).T),
            "bk2": np.ascontiguousarray(bk[gs].reshape(2, 128).T),
            "bvr": np.ascontiguousarray(bv[gs].reshape(1, GW)),
        })
    return in_maps


def run_on_hw(in_maps, trace=False, **kw):
    nc = _build_program()
    return run_bass_kernel_spmd(nc, in_maps, core_ids=list(range(N_CORES)),
                                trace=trace, **kw)


def kernel(query, key, value, Wq, bq, Wk, bk, Wv, bv, Wo, bo):
    in_maps = make_in_maps(query, key, value, Wq, bq, Wk, bk, Wv, bv, Wo, bo)
    res = run_on_hw(in_maps)
    out = np.zeros((B, S, D_MODEL), np.float32)
    for c in range(N_CORES):
        out[c // GROUPS] += res.results[c]["out"].astype(np.float32)
    out += np.asarray(bo, np.float32)
    return out



# revision 43
# speedup vs baseline: 1.0295x; 1.0182x over previous
"""Multi-head attention (B=2, S=2048, D=1024, H=16) on 8 Trainium2 NeuronCores.

Sharding: tensor-parallel on heads (4 groups of 4 heads) x data-parallel on
batch (2) -> 8 cores. Each core computes QKV projections for its head slice,
attention for its 4 heads, and a partial output projection; the host sums the
4 partials per batch element (the tensor-parallel allreduce) and adds bo.

v3 design: the softmax exp stream on the ACT engine is the pacing resource
(~135us); everything else hides under it.
- One flat stream of (head, q-block, kblock) steps: scores -> exp each step,
  attn@V lagging 3 steps behind, so head boundaries never stall the exp
  stream (the in-order PE queue always has ready work in front).
- Scores are computed transposed (ST[k,q] = K_h.T @ Q_h) per (head, kblock)
  into a 2-buffer x [128,1024] PSUM ring (4 banks) -> exp runs at FD=1024
  with no ping-pong stalls.  The two 512-wide score matmuls of each step run
  CONCURRENTLY on disjoint 64-row PE tiles (K/Q live in both partition
  halves via a partition-swapped copy), so dk=64 wastes nothing.
- attn@V accumulates po[65, 1024] in PSUM (2 banks); a ones-column appended
  to V makes row 64 the softmax denominators.  Normalize first copies po to
  SBUF (freeing the single po buffer early), then recip/broadcast/scale.
- QKV/O projections are [128,512]-output units on a 2-buffer PSUM pool
  (2 banks), interleaved into the stream to fill PE slack.  Prologue DMAs
  are spread across the sync/scalar/gpsimd queues (the ~625ns per-DMA issue
  cost on one queue would otherwise serialize the start).
"""

import numpy as np

import concourse.bass as bass  # noqa: F401
import concourse.tile as tile
from concourse import bacc, mybir
from concourse.bass_utils import run_bass_kernel_spmd

D_MODEL = 1024
NUM_HEADS = 16
DK = 64
B, S = 2, 2048
N_CORES = 8
GROUPS = 4                 # head groups (tensor parallel)
GW = D_MODEL // GROUPS     # 256 features per group = 4 heads
HPG = GROUPS               # heads per group = 4

F32 = mybir.dt.float32
F16 = mybir.dt.float16
EXPF = mybir.ActivationFunctionType.Exp
MULT = mybir.AluOpType.mult
ADD = mybir.AluOpType.add

LAG = 3                    # attn@V trails exp by this many stream steps


def _emit(nc, tc, ctx):
    P = 128
    xqT = nc.dram_tensor("xqT", [D_MODEL, S], F16, kind="ExternalInput")
    xkT = nc.dram_tensor("xkT", [D_MODEL, S], F16, kind="ExternalInput")
    xvT = nc.dram_tensor("xvT", [D_MODEL, S], F16, kind="ExternalInput")
    wqT = nc.dram_tensor("wqT", [D_MODEL, GW], F16, kind="ExternalInput")
    wkT = nc.dram_tensor("wkT", [D_MODEL, GW], F16, kind="ExternalInput")
    wvT = nc.dram_tensor("wvT", [D_MODEL, GW], F16, kind="ExternalInput")
    woT = nc.dram_tensor("woT", [GW, D_MODEL], F16, kind="ExternalInput")
    bq2 = nc.dram_tensor("bq2", [P, 2], F32, kind="ExternalInput")
    bk2 = nc.dram_tensor("bk2", [P, 2], F32, kind="ExternalInput")
    bvr = nc.dram_tensor("bvr", [1, GW], F32, kind="ExternalInput")
    out = nc.dram_tensor("out", [S, D_MODEL], F16, kind="ExternalOutput")

    consts = ctx.enter_context(tc.tile_pool(name="consts", bufs=1))
    persist = ctx.enter_context(tc.tile_pool(name="persist", bufs=1))
    xs = ctx.enter_context(tc.tile_pool(name="xs", bufs=3))
    xv_pool = ctx.enter_context(tc.tile_pool(name="xvs", bufs=3))
    sx = ctx.enter_context(tc.tile_pool(name="stexp", bufs=5))
    nrm = ctx.enter_context(tc.tile_pool(name="nrm", bufs=1))
    outp = ctx.enter_context(tc.tile_pool(name="outp", bufs=3))
    o1pool = ctx.enter_context(tc.tile_pool(name="o1pool", bufs=8))
    psA = ctx.enter_context(tc.tile_pool(name="psA", bufs=2, space="PSUM"))
    psB = ctx.enter_context(tc.tile_pool(name="psB", bufs=1, space="PSUM"))
    psP = ctx.enter_context(tc.tile_pool(name="psP", bufs=2, space="PSUM"))

    # ---- weights / constants (queue-balanced) ---------------------------
    wq_sb = consts.tile([P, 8, GW], F16)
    wk_sb = consts.tile([P, 8, GW], F16)
    wv_sb = consts.tile([P, 8, GW], F16)
    wo_sb = consts.tile([P, 2, D_MODEL], F16)
    bq_sb = consts.tile([P, 2], F32)
    bk_sb = consts.tile([P, 2], F32)
    bv_row = consts.tile([1, GW], F32)
    nc.sync.dma_start(wq_sb[:], wqT[:].rearrange("(c p) j -> p c j", p=P))
    nc.sync.dma_start(bq_sb[:], bq2[:])
    nc.scalar.dma_start(wk_sb[:], wkT[:].rearrange("(c p) j -> p c j", p=P))
    nc.scalar.dma_start(bk_sb[:], bk2[:])
    nc.gpsimd.dma_start(bv_row[:], bvr[:])
    bvb = consts.tile([P, GW], F32)
    nc.gpsimd.partition_broadcast(bvb[:], bv_row[:])

    # persistent activations.  QT[jc]: projected Q^T, partitions = the 128
    # features of chunk jc (head (jc,0) rows 0-63, head (jc,1) rows 64-127),
    # free dim = q.  QTw = partition-halves-swapped copy (for PE row-tiling).
    # After attention, OT overwrites QT's dead region (per-head rows).
    QT = [persist.tile([P, S], F16, name=f"QT{j}") for j in range(2)]
    QTw = [persist.tile([P, S], F16, name=f"QTw{j}") for j in range(2)]
    KT = [[persist.tile([P, 1024], F16, name=f"KT{j}_{sb}") for sb in range(2)]
          for j in range(2)]
    KTw = [[persist.tile([P, 1024], F16, name=f"KTw{j}_{sb}") for sb in range(2)]
           for j in range(2)]
    # V + ones column, per (kblock, head): lhsT of attn@V
    Vaug = persist.tile([P, 16, HPG, DK + 1], F16, name="Vaug")
    ones_f32 = consts.tile([P, 16, HPG], F32)
    nc.vector.memset(ones_f32[:], 1.0)
    # warm the ACT exp table during the DMA-bound prologue
    warm = consts.tile([1, 1], F32)
    nc.scalar.activation(out=warm[:], in_=ones_f32[0:1, 0, 0:1], func=EXPF)
    nc.vector.tensor_scalar_add(Vaug[:, :, :, DK], ones_f32[:], 0.0)

    xqT_r = xqT[:].rearrange("(c p) s -> p c s", p=P)
    xkT_r = xkT[:].rearrange("(c p) s -> p c s", p=P)
    xvT_r = xvT[:].rearrange("(c p) s -> p c s", p=P)

    # ---- projection work units ------------------------------------------
    def stage_x(x_r, sb, name, eng=None, split=2):
        """Stage one s-half of an input: 8 [128,1024] chunks in `split` DMAs."""
        eng = eng or nc.sync
        xt = xs.tile([P, 8, 1024], F16, tag="xs", name=f"x{name}{sb}")
        step = 8 // split
        for g in range(split):
            eng.dma_start(xt[:, g * step:(g + 1) * step, :],
                          x_r[:, g * step:(g + 1) * step,
                              sb * 1024:(sb + 1) * 1024])
        return xt

    def qk_mms(xt, w_sb, b_sb, dst, jc, sb, ns, full_s=False):
        """Projection matmuls + main drain for 512 s-columns of chunk jc.
        Returns a closure writing the partition-swapped copy (reads the same
        PSUM tile, so it must run before psP recycles this buffer)."""
        ps = psP.tile([P, 512], F32, tag="psP", name=f"ps{jc}{sb}{ns}")
        for i in range(8):
            nc.tensor.matmul(
                ps[:, :],
                w_sb[:, i, jc * P:(jc + 1) * P],
                xt[:, i, ns * 512:(ns + 1) * 512],
                start=(i == 0), stop=(i == 7),
            )
        c0 = (sb * 1024 if full_s else 0) + ns * 512
        sl = slice(c0, c0 + 512)
        nc.vector.tensor_scalar_add(dst[:, sl], ps[:], b_sb[:, jc:jc + 1])

        def swaps(dstw):
            nc.vector.tensor_scalar_add(dstw[64:128, sl], ps[0:64, :],
                                        b_sb[0:64, jc:jc + 1])
            nc.vector.tensor_scalar_add(dstw[0:64, sl], ps[64:128, :],
                                        b_sb[64:128, jc:jc + 1])
        return swaps

    def qk_unit(xt, w_sb, b_sb, dst, dstw, jc, sb, ns, full_s=False):
        qk_mms(xt, w_sb, b_sb, dst, jc, sb, ns, full_s)(dstw)

    def v_unit(kb, staged=None):
        """V projection for kblock kb -> Vaug[:, kb, :, 0:64] (+bias)."""
        if staged is None:
            xvt = xv_pool.tile([P, 8, P], F16, tag="xv", name=f"xv{kb}")
            nc.gpsimd.dma_start(xvt[:], xvT_r[:, :, kb * P:(kb + 1) * P])
            src_ap = xvt
            c0 = 0
        else:
            src_ap = staged
            c0 = (kb % 4) * P
        pv = psP.tile([P, 512], F32, tag="psP", name=f"pv{kb}")
        for i in range(8):
            nc.tensor.matmul(
                pv[:, 0:GW], src_ap[:, i, c0:c0 + P], wv_sb[:, i, :],
                start=(i == 0), stop=(i == 7),
            )
        nc.vector.tensor_tensor(
            Vaug[:, kb, :, 0:DK],
            pv[:, 0:GW].rearrange("p (h d) -> p h d", h=HPG),
            bvb[:].rearrange("p (h d) -> p h d", h=HPG),
            ADD,
        )

    ot_tiles = {}

    def oproj_full(sc):
        """Complete O projection for s-chunk sc (both halves + store)."""
        ot = outp.tile([P, 1024], F16, tag="osb", name=f"ot{sc}")
        for ms in range(2):
            pso = psP.tile([P, 512], F32, tag="psP", name=f"pso{sc}{ms}")
            for hd in range(2):
                nc.tensor.matmul(
                    pso[:, :],
                    QT[hd][:, sc * P:(sc + 1) * P],
                    wo_sb[:, hd, ms * 512:(ms + 1) * 512],
                    start=(hd == 0), stop=(hd == 1),
                )
            nc.vector.tensor_copy(out=ot[:, ms * 512:(ms + 1) * 512],
                                  in_=pso[:, :])
        eng = nc.sync if sc % 2 == 0 else nc.gpsimd
        eng.dma_start(out[sc * P:(sc + 1) * P, :], ot[:])

    def oproj_part1(sc):
        """hd=0 half of the contraction for s-chunk sc -> SBUF staging."""
        ot = o1pool.tile([P, 1024], F32, tag="o1", name=f"o1_{sc}")
        ot_tiles[sc] = ot
        for ms in range(2):
            pso = psP.tile([P, 512], F32, tag="psP", name=f"p1_{sc}{ms}")
            nc.tensor.matmul(pso[:, :], QT[0][:, sc * P:(sc + 1) * P],
                             wo_sb[:, 0, ms * 512:(ms + 1) * 512],
                             start=True, stop=True)
            nc.vector.tensor_copy(out=ot[:, ms * 512:(ms + 1) * 512],
                                  in_=pso[:, :])

    def oproj_part2(sc):
        """hd=1 half + add + store for s-chunk sc."""
        ot1 = ot_tiles[sc]
        ot2 = outp.tile([P, 1024], F16, tag="osb2", name=f"o2_{sc}")
        for ms in range(2):
            pso = psP.tile([P, 512], F32, tag="psP", name=f"p2_{sc}{ms}")
            nc.tensor.matmul(pso[:, :], QT[1][:, sc * P:(sc + 1) * P],
                             wo_sb[:, 1, ms * 512:(ms + 1) * 512],
                             start=True, stop=True)
            nc.vector.tensor_tensor(ot2[:, ms * 512:(ms + 1) * 512], pso[:, :],
                                    ot1[:, ms * 512:(ms + 1) * 512], ADD)
        eng = nc.sync if sc % 2 == 0 else nc.gpsimd
        eng.dma_start(out[sc * P:(sc + 1) * P, :], ot2[:])

    # ---- attention pieces ----------------------------------------------
    _pst_box = [None]

    def scores_pair(h, qb, kb):
        """ST[k,q]: two concurrent 64-row-tile matmuls (home + swapped)."""
        jc, hp = h // 2, h % 2
        pr = 64 * hp
        prw = 64 - pr
        sb, kc = kb // 8, (kb % 8) * P
        pst = psA.tile([P, 1024], F32, tag="psA", name=f"pst{h}{qb}{kb}")
        _pst_box[0] = pst
        q0 = qb * 1024
        nc.tensor.matmul(
            pst[:, 0:512],
            KT[jc][sb][pr:pr + DK, kc:kc + P],
            QT[jc][pr:pr + DK, q0:q0 + 512],
            start=True, stop=True,
        )
        nc.tensor.matmul(
            pst[:, 512:1024],
            KTw[jc][sb][prw:prw + DK, kc:kc + P],
            QTw[jc][prw:prw + DK, q0 + 512:q0 + 1024],
            start=True, stop=True,
        )

    def attn_v(po, st, h, kb):
        for ns in range(2):
            nc.tensor.matmul(
                po[0:DK + 1, ns * 512:(ns + 1) * 512],
                Vaug[:, kb, h, :],
                st[:, ns * 512:(ns + 1) * 512],
                start=(kb == 0), stop=(kb == 15),
            )

    def normalize(po, h, qb):
        """Copy po out of PSUM (frees the single po buffer), then scale by
        the reciprocal of the denominators (row DK) and write OT into QT."""
        jc, hp = h // 2, h % 2
        pr = 64 * hp
        poc = nrm.tile([DK + 1, 1024], F32, tag="poc", name=f"poc{h}{qb}")
        dn = nrm.tile([1, 1024], F32, tag="dn", name=f"dn{h}{qb}")
        nc.vector.tensor_copy(out=poc[:], in_=po[0:DK + 1, :])
        nc.vector.tensor_copy(out=dn[:], in_=poc[DK:DK + 1, :])
        bc = nrm.tile([DK, 1024], F32, tag="bcast", name=f"bc{h}{qb}")
        nc.vector.reciprocal_approx_fast(bc[0:1, :], dn[:])
        nc.gpsimd.partition_broadcast(bc[:], bc[0:1, :])
        q0 = qb * 1024
        nc.vector.tensor_tensor(
            QT[jc][pr:pr + DK, q0:q0 + 1024], poc[0:DK, :], bc[:], MULT)

    # ---- prologue -------------------------------------------------------
    # xq0 on sync, xk0 on scalar (idle until the first exp), weights split.
    xq0 = stage_x(xqT_r, 0, "q", eng=nc.sync, split=2)
    xk0 = stage_x(xkT_r, 0, "k", eng=nc.scalar, split=2)
    dum = consts.tile([P, 512], F16)
    nc.vector.memset(dum[:], 0.001)
    warm_ps = psA.tile([P, 1024], F32, tag="psA", name="warmps")
    for _ in range(14):
        nc.tensor.matmul(warm_ps[:, 0:512], dum[:, 0:128], dum[:, :],
                         start=True, stop=True)
    sw_q0 = qk_mms(xq0, wq_sb, bq_sb, QT[0], 0, 0, 0, full_s=True)
    sw_q1 = qk_mms(xq0, wq_sb, bq_sb, QT[0], 0, 0, 1, full_s=True)
    sw_q0(QTw[0])
    sw_q1(QTw[0])
    sw_k0 = qk_mms(xk0, wk_sb, bk_sb, KT[0][0], 0, 0, 0)
    sw_k0(KTw[0][0])
    sw_k1 = qk_mms(xk0, wk_sb, bk_sb, KT[0][0], 0, 0, 1)
    sw_k1(KTw[0][0])
    wvT_r = wvT[:].rearrange("(c p) j -> p c j", p=P)
    nc.gpsimd.dma_start(wv_sb[:], wvT_r)
    xv03 = consts.tile([P, 8, 512], F16)
    nc.sync.dma_start(xv03[:], xvT_r[:, :, 0:512])
    xv47 = consts.tile([P, 8, 512], F16)
    nc.sync.dma_start(xv47[:], xvT_r[:, :, 512:1024])
    woT_r = woT[:].rearrange("(c p) m -> p c m", p=P)
    for cc in range(2):
        nc.sync.dma_start(wo_sb[:, cc, :], woT_r[:, cc, :])
    for kb in range(4):
        v_unit(kb, xv03)
    xk1 = stage_x(xkT_r, 1, "k1", eng=nc.scalar, split=2)

    # deferred projection / oproj work, as (slot -> closure) over the stream
    box = {}
    fills = {}

    def put(step, fn):
        fills.setdefault(step, []).append(fn)

    # u0 (h0,qb0): remaining V units inline + the sb1 K units it needs
    for kb in range(12):
        put(kb, lambda kb=kb: v_unit(kb + 4, xv47 if kb < 4 else None))
    put(1, lambda: qk_unit(xk1, wk_sb, bk_sb, KT[0][1], KTw[0][1], 0, 1, 0))
    put(4, lambda: qk_unit(xk1, wk_sb, bk_sb, KT[0][1], KTw[0][1], 0, 1, 1))
    # u1 (h1,qb0): Q/K for the jc=1 heads + stage xq sb1
    put(17, lambda: qk_unit(xq0, wq_sb, bq_sb, QT[1], QTw[1], 1, 0, 0,
                            full_s=True))
    put(19, lambda: qk_unit(xq0, wq_sb, bq_sb, QT[1], QTw[1], 1, 0, 1,
                            full_s=True))
    put(21, lambda: qk_unit(xk0, wk_sb, bk_sb, KT[1][0], KTw[1][0], 1, 0, 0))
    put(23, lambda: qk_unit(xk0, wk_sb, bk_sb, KT[1][0], KTw[1][0], 1, 0, 1))
    put(26, lambda: box.__setitem__(
        "xq1", stage_x(xqT_r, 1, "q1", eng=nc.gpsimd, split=2)))
    # u2 (h2,qb0): jc=1 sb1 K units + Q sb1 units
    put(33, lambda: qk_unit(xk1, wk_sb, bk_sb, KT[1][1], KTw[1][1], 1, 1, 0))
    put(35, lambda: qk_unit(xk1, wk_sb, bk_sb, KT[1][1], KTw[1][1], 1, 1, 1))
    put(38, lambda: qk_unit(box["xq1"], wq_sb, bq_sb, QT[0], QTw[0], 0, 1, 0,
                            full_s=True))
    put(41, lambda: qk_unit(box["xq1"], wq_sb, bq_sb, QT[0], QTw[0], 0, 1, 1,
                            full_s=True))
    put(44, lambda: qk_unit(box["xq1"], wq_sb, bq_sb, QT[1], QTw[1], 1, 1, 0,
                            full_s=True))
    put(47, lambda: qk_unit(box["xq1"], wq_sb, bq_sb, QT[1], QTw[1], 1, 1, 1,
                            full_s=True))
    # qb0 O-projection during u4,u5 (needs all qb0 heads normalized)
    for j, sc in enumerate(range(8)):
        put(68 + 3 * j, lambda sc=sc: oproj_full(sc))
    # qb1 O-projection: hd0 half during u6/u7, hd1 half in the tail
    for j, sc in enumerate(range(8, 16)):
        put(99 + 3 * j, lambda sc=sc: oproj_part1(sc))

    # ---- the flat attention stream -------------------------------------
    units = [(h, qb) for qb in (0, 1) for h in range(4)]
    seq = [(u, kb) for u in range(8) for kb in range(16)]
    po_of = {}
    sts = {}

    def do_lagged(i):
        u2, kb2 = seq[i]
        h2, qb2 = units[u2]
        if kb2 == 0:
            po_of[u2] = psB.tile([P, 1024], F32, tag="po", name=f"po{u2}")
        attn_v(po_of[u2], sts[(u2, kb2)], h2, kb2)
        del sts[(u2, kb2)]
        if kb2 == 15:
            normalize(po_of[u2], h2, qb2)

    for i, (u, kb) in enumerate(seq):
        h, qb = units[u]
        scores_pair(h, qb, kb)
        st = sx.tile([P, 1024], F16, tag="st", name=f"st{u}_{kb}")
        nc.scalar.activation(out=st[:], in_=_pst_box[0][:], func=EXPF,
                             scale=0.125)
        sts[(u, kb)] = st
        if i >= LAG:
            do_lagged(i - LAG)
        for fn in fills.pop(i, ()):
            fn()
    for i in range(len(seq) - LAG, len(seq)):
        do_lagged(i)
    # tail: finish qb1 O-projection
    for sc in range(8, 16):
        oproj_part2(sc)


_prog_cache = {}


def _build_program():
    if "nc" not in _prog_cache:
        from contextlib import ExitStack
        nc = bacc.Bacc("TRN2", target_bir_lowering=False)
        with tile.TileContext(nc) as tc:
            with ExitStack() as ctx:
                _emit(nc, tc, ctx)
        nc.compile()
        _prog_cache["nc"] = nc
    return _prog_cache["nc"]


def make_in_maps(query, key, value, Wq, bq, Wk, bk, Wv, bv, Wo, bo):
    query, key, value = (np.asarray(t, np.float32) for t in (query, key, value))
    Wq, Wk, Wv, Wo = (np.asarray(t, np.float32) for t in (Wq, Wk, Wv, Wo))
    bq, bk, bv = (np.asarray(t, np.float32) for t in (bq, bk, bv))
    xT = {b: {} for b in range(B)}
    for b in range(B):
        xT[b]["q"] = np.ascontiguousarray(query[b].T).astype(np.float16)
        xT[b]["k"] = np.ascontiguousarray(key[b].T).astype(np.float16)
        xT[b]["v"] = np.ascontiguousarray(value[b].T).astype(np.float16)
    in_maps = []
    for c in range(N_CORES):
        b, g = divmod(c, GROUPS)
        gs = slice(g * GW, (g + 1) * GW)
        in_maps.append({
            "xqT": xT[b]["q"], "xkT": xT[b]["k"], "xvT": xT[b]["v"],
            "wqT": np.ascontiguousarray(Wq[gs, :].T).astype(np.float16),
            "wkT": np.ascontiguousarray(Wk[gs, :].T).astype(np.float16),
            "wvT": np.ascontiguousarray(Wv[gs, :].T).astype(np.float16),
            "woT": np.ascontiguousarray(Wo[:, gs].T).astype(np.float16),
            "bq2": np.ascontiguousarray(bq[gs].reshape(2, 128).T),
            "bk2": np.ascontiguousarray(bk[gs].reshape(2, 128).T),
            "bvr": np.ascontiguousarray(bv[gs].reshape(1, GW)),
        })
    return in_maps


def run_on_hw(in_maps, trace=False, **kw):
    nc = _build_program()
    return run_bass_kernel_spmd(nc, in_maps, core_ids=list(range(N_CORES)),
                                trace=trace, **kw)


def kernel(query, key, value, Wq, bq, Wk, bk, Wv, bv, Wo, bo):
    in_maps = make_in_maps(query, key, value, Wq, bq, Wk, bk, Wv, bv, Wo, bo)
    res = run_on_hw(in_maps)
    out = np.zeros((B, S, D_MODEL), np.float32)
    for c in range(N_CORES):
        out[c // GROUPS] += res.results[c]["out"].astype(np.float32)
    out += np.asarray(bo, np.float32)
    return out

